# revision 30
# baseline (speedup 1.0000x reference)
"""Trainium2 Bass kernel for the Enigma-style CopyMemoryModel.

Math (validated vs reference):
  - The lax.scan carries nothing -> every timestep t is independent.
  - t < 128 and d = 1024  =>  rotors 1,2 have pos = 0 (no roll); only rotor 0
    rolls by t, and roll(roll(h,-t) @ W, t) == h @ roll(W, (t,t), (0,1)).
  - Everything before the first rev block is LINEAR with the only t-dependence
    being rotor 0's roll -> fold on host into per-t head matrices
        Mhead_t = [P@Wi | P@bi].T @ roll(rotW0,(t,t)) @ (rotW1@rotW2)   [65,1024]
  - Everything after the last rev block is linear too -> per-t tail matrices
        Mtail_t = (rotW1@rotW2).T @ roll(rotW0,(t,t)).T @ (P@Wo.T)      [1024,64]
  - On chip only:  head (per-t small matmuls), 6 rev couplings fwd,
    Srefl = R+R.T big stage, 6 rev couplings bwd, tail.  bo added on host.
  - Layout on chip: activations stored transposed, hT[128 part, 8 blocks x 1024
    tokens] per core; every stage is out_block[jt] = sum_kt W[kt,jt].T @ h[kt].
  - bf16 datapath (PSUM accumulation fp32, final output fp32).

Perf structure (v2):
  - All input DMAs hoisted to kernel start, spread over 4 issue queues in
    consumption order: scalar{xt,wf0,wg0,wf1,wg1}, sync{mh even, ws x4},
    vector{mh 1,3,5,7}, gpsimd{mhb, mh 9,11,13,15, wf2, wg2, mt x4}.
    In-queue ordering delays ws/mt transfers behind the critical head DMAs.
  - Srefl weights stored jt-major so each output block jt only needs its
    1024-col slice -> Srefl can start before the full 2MB matrix lands.
  - Small junk-matmul warmup keeps the PE p-state ramp warm while the first
    DMAs land.

Sharding: time-sharded; core c handles t in [c*16, (c+1)*16), all 64 batch
samples -> 1024 tokens per core, token column = g*64 + b.
"""
import numpy as np

B, S, DIN, D, DOUT = 64, 128, 64, 1024, 64
NCORES = 8
TLOC = S // NCORES          # 16 timesteps per core
NTOK = B * TLOC             # 1024 tokens per core
NB = D // 128               # 8 row blocks
NCH = NTOK // 512           # 2 column chunks of 512
HB = 512                    # half of D (rev-block split)
NBH = HB // 128             # 4 blocks per half

_compiled = {}


def _build():
    import concourse.bacc as bacc
    import concourse.mybir as mybir
    from concourse.tile import TileContext

    f32 = mybir.dt.float32
    bf16 = mybir.dt.bfloat16
    ACT_TANH = mybir.ActivationFunctionType.Tanh
    ACT_COPY = mybir.ActivationFunctionType.Copy

    nc = bacc.Bacc(None, target_bir_lowering=False, debug=True)

    xt_d = nc.dram_tensor("xt", [DIN, NTOK], bf16, kind="ExternalInput")
    mh_d = nc.dram_tensor("mhead", [DIN, TLOC * D], bf16, kind="ExternalInput")
    mhb_d = nc.dram_tensor("mheadb", [128, TLOC * NB], bf16, kind="ExternalInput")
    wf_d = nc.dram_tensor("wf", [3, 128, NBH * HB], bf16, kind="ExternalInput")
    wg_d = nc.dram_tensor("wg", [3, 128, NBH * HB], bf16, kind="ExternalInput")
    ws_d = nc.dram_tensor("wsrefl", [128, NB * D], bf16, kind="ExternalInput")
    mt_d = nc.dram_tensor("mtail", [128, TLOC * HB], bf16, kind="ExternalInput")
    out_d = nc.dram_tensor("out", [DOUT, NTOK], f32, kind="ExternalOutput")

    with TileContext(nc) as tc:
        with (
            tc.tile_pool(name="hbuf", bufs=1) as hpool,
            tc.tile_pool(name="wpool", bufs=1) as wpool,
            tc.tile_pool(name="fgpool", bufs=1) as fgpool,
            tc.tile_pool(name="hdpool", bufs=1) as hdpool,
            tc.tile_pool(name="cpool", bufs=1) as cpool,
            tc.tile_pool(name="tpool", bufs=3) as tpool,
            tc.tile_pool(name="ps1", bufs=6, space="PSUM") as ps1,
            tc.tile_pool(name="psw", bufs=2, space="PSUM") as psw,
        ):
            hA = hpool.tile([128, NB * NTOK], bf16)
            hB = hpool.tile([128, NB * NTOK], bf16)
            hAR = hA[:].rearrange("p (n t) -> p n t", n=NB)

            xt = cpool.tile([DIN, NTOK], bf16)
            bT = cpool.tile([128, TLOC * NB], bf16)
            outsb = cpool.tile([DOUT, NTOK], f32)

            junk = cpool.tile([128, 256], bf16)
            nc.gpsimd.memset(junk[:], 0.0)

            def junk_mm(n, gate=None):
                for r in range(n):
                    wps = psw.tile([128, 256], f32, tag="sm")
                    if r == 0 and gate is not None:
                        # pace the PE start to DMA supply: first junk reads a
                        # mid-schedule DMA's landing zone, so the PE starts
                        # late but then runs gap-free (keeps p-state high)
                        nc.tensor.matmul(wps[:], junk[0:64, 0:128],
                                         gate, start=True, stop=True)
                    else:
                        nc.tensor.matmul(wps[:], junk[:, 0:128],
                                         junk[:], start=True, stop=True)

            # ---- hoisted input DMAs, 4 issue queues, consumption order ----
            mhall = hdpool.tile([DIN, TLOC * D], bf16)
            wfT = [fgpool.tile([128, NBH * HB], bf16, tag=f"wf{i}",
                               name=f"wf{i}") for i in range(3)]
            wgT = [fgpool.tile([128, NBH * HB], bf16, tag=f"wg{i}",
                               name=f"wg{i}") for i in range(3)]
            wsT = wpool.tile([128, NB * D], bf16, tag="ws")
            mtT = wpool.tile([128, TLOC * HB], bf16, tag="mt")

            # ~128-256KB pieces over the 3 DGE queues in consumption order.
            # Only the head-critical set issues up front; everything needed
            # later is gated behind compute milestones (see `gate` below) so
            # the scheduler cannot hoist it into the critical window.
            def mh_piece(q, g):
                q.dma_start(mhall[0:DIN, g * D:(g + 1) * D],
                            mh_d[:, g * D:(g + 1) * D])

            def half(q, dst, src, h):
                q.dma_start(dst[:, h * 1024:(h + 1) * 1024],
                            src[:, h * 1024:(h + 1) * 1024])

            # scalar (HW ring; must finish issuing before first tanh)
            nc.scalar.dma_start(xt[:], xt_d[:])
            mh_piece(nc.scalar, 1)
            mh_piece(nc.scalar, 3)
            half(nc.scalar, wfT[0][:], wf_d[0], 1)
            mh_piece(nc.scalar, 6)
            mh_piece(nc.scalar, 9)
            mh_piece(nc.scalar, 12)
            mh_piece(nc.scalar, 15)
            # sync (HW ring)
            nc.sync.dma_start(bT[:], mhb_d[:])
            mh_piece(nc.sync, 0)
            mh_piece(nc.sync, 2)
            mh_piece(nc.sync, 4)
            half(nc.sync, wfT[0][:], wf_d[0], 0)
            mh_piece(nc.sync, 7)
            mh_piece(nc.sync, 10)
            mh_piece(nc.sync, 13)
            # gpsimd (SW DGE, slower — fewer pieces)
            mh_piece(nc.gpsimd, 5)
            half(nc.gpsimd, wgT[0][:], wg_d[0], 0)
            half(nc.gpsimd, wgT[0][:], wg_d[0], 1)
            mh_piece(nc.gpsimd, 8)
            mh_piece(nc.gpsimd, 11)
            mh_piece(nc.gpsimd, 14)

            def gate(dst_tile, col, src_tile, scol):
                # tiny WAW anchor: the following DMA into dst_tile[:, col:...]
                # cannot issue until src_tile[0, scol:scol+2] is final
                nc.gpsimd.tensor_copy(dst_tile[0:1, col:col + 2],
                                      src_tile[0:1, scol:scol + 2])

            with nc.named_scope("warmup"):
                junk_mm(4, gate=mhall[0:64, 0:256])

            # head: h[jt-block, g-tokens] = Mhead_t[:, jt].T @ x[g-tokens] + b
            def head_group(g):
                mo = g * D
                gs, ge = g * B, (g + 1) * B
                ps = ps1.tile([128, 512], f32)
                for jt in range(NB):
                    nc.tensor.matmul(ps[:, jt * 64:(jt + 1) * 64],
                                     mhall[:, mo + jt * 128:mo + (jt + 1) * 128],
                                     xt[:, gs:ge], start=True, stop=True)
                psR = ps[:].rearrange("p (n t) -> p n t", n=NB)
                bias = bT[:, g * NB:(g + 1) * NB].broadcast_to([128, NB, B])
                nc.vector.tensor_tensor(hAR[:, :, gs:ge], psR, bias,
                                        op=mybir.AluOpType.add)

            def big_stage(src, dst, w):
                # jt-major weight layout: w[:, jt*D + kt*128 + m]
                for ch in range(NCH):
                    for jt in range(NB):
                        ps = ps1.tile([128, 512], f32)
                        for kt in range(NB):
                            nc.tensor.matmul(
                                ps[:],
                                w[:, jt * D + kt * 128:jt * D + (kt + 1) * 128],
                                src[:, kt * NTOK + ch * 512:kt * NTOK + (ch + 1) * 512],
                                start=(kt == 0), stop=(kt == NB - 1),
                            )
                        nc.scalar.activation(
                            dst[:, jt * NTOK + ch * 512:jt * NTOK + (ch + 1) * 512],
                            ps[:], ACT_COPY)

            def coupling_chunk(buf, fg, in_half, out_half, ch):
                # buf[out_half] += tanh(W.T @ buf[in_half]) for token chunk ch
                # jt-major weight layout: fg[:, jt*HB + kt*128 + m]
                for jt in range(NBH):
                    ps = ps1.tile([128, 512], f32)
                    for kt in range(NBH):
                        nc.tensor.matmul(
                            ps[:],
                            fg[:, jt * HB + kt * 128:jt * HB + (kt + 1) * 128],
                            buf[:, (in_half * NBH + kt) * NTOK + ch * 512:
                                (in_half * NBH + kt) * NTOK + (ch + 1) * 512],
                            start=(kt == 0), stop=(kt == NBH - 1),
                        )
                    tmp = tpool.tile([128, 512], bf16)
                    nc.scalar.activation(tmp[:], ps[:], ACT_TANH)
                    dsl = buf[:, (out_half * NBH + jt) * NTOK + ch * 512:
                              (out_half * NBH + jt) * NTOK + (ch + 1) * 512]
                    nc.vector.tensor_add(dsl, dsl, tmp[:])

            def coupling(buf, fg, in_half, out_half):
                for ch in range(NCH):
                    coupling_chunk(buf, fg, in_half, out_half, ch)

            def rev_block(buf, i):
                coupling(buf, wfT[i][:], in_half=1, out_half=0)
                coupling(buf, wgT[i][:], in_half=0, out_half=1)

            def scoped(name, fn, *args, **kw):
                with nc.named_scope(name):
                    fn(*args, **kw)

            # junk filler between head groups: keeps the PE busy-streak (and
            # p-state) alive while the group DMAs trickle in
            with nc.named_scope("head"):
                for g in range(8):
                    head_group(g)
                    junk_mm(1)
            scoped("revf0a", coupling_chunk, hA, wfT[0][:], 1, 0, 0)
            with nc.named_scope("head2"):
                for g in range(8, TLOC):
                    head_group(g)
                    if g < 14:
                        junk_mm(1)
            # wf1 after revf0a (anchor: hA half0 ch0, written by revf0a)
            gate(wfT[1], 0, hA, 0)
            half(nc.sync, wfT[1][:], wf_d[1], 0)
            gate(wfT[1], 1024, hA, 2)
            half(nc.gpsimd, wfT[1][:], wf_d[1], 1)
            scoped("revf0b", coupling_chunk, hA, wfT[0][:], 1, 0, 1)
            # wg1 after revf0b (anchor: hA half0 ch1)
            gate(wgT[1], 0, hA, 512)
            half(nc.sync, wgT[1][:], wg_d[1], 0)
            gate(wgT[1], 1024, hA, 514)
            half(nc.gpsimd, wgT[1][:], wg_d[1], 1)
            scoped("revf0c", coupling, hA, wgT[0][:], 0, 1)
            # wf2 after revf0c (anchor: hA half1)
            gate(wfT[2], 0, hA, 4 * NTOK)
            half(nc.sync, wfT[2][:], wf_d[2], 0)
            gate(wfT[2], 1024, hA, 4 * NTOK + 2)
            half(nc.gpsimd, wfT[2][:], wf_d[2], 1)
            scoped("revf1F", coupling, hA, wfT[1][:], 1, 0)
            # wg2 + ws q0 after revf1F
            gate(wgT[2], 0, hA, 4)
            half(nc.sync, wgT[2][:], wg_d[2], 0)
            gate(wgT[2], 1024, hA, 6)
            half(nc.gpsimd, wgT[2][:], wg_d[2], 1)
            gate(wsT, 0, hA, 8)
            nc.sync.dma_start(wsT[:, 0:2048], ws_d[:, 0:2048])
            scoped("revf1G", coupling, hA, wgT[1][:], 0, 1)
            # ws q1/q2 after revf1G
            gate(wsT, 2048, hA, 4 * NTOK + 4)
            nc.sync.dma_start(wsT[:, 2048:4096], ws_d[:, 2048:4096])
            gate(wsT, 4096, hA, 4 * NTOK + 6)
            nc.gpsimd.dma_start(wsT[:, 4096:6144], ws_d[:, 4096:6144])
            scoped("revf2F", coupling, hA, wfT[2][:], 1, 0)
            # ws q3 + mt q0/q1 after revf2F
            gate(wsT, 6144, hA, 10)
            nc.sync.dma_start(wsT[:, 6144:8192], ws_d[:, 6144:8192])
            gate(mtT, 0, hA, 12)
            nc.gpsimd.dma_start(mtT[:, 0:2048], mt_d[:, 0:2048])
            gate(mtT, 2048, hA, 14)
            nc.sync.dma_start(mtT[:, 2048:4096], mt_d[:, 2048:4096])
            scoped("revf2G", coupling, hA, wgT[2][:], 0, 1)
            # mt q2/q3 after revf2G
            gate(mtT, 4096, hA, 4 * NTOK + 8)
            nc.gpsimd.dma_start(mtT[:, 4096:6144], mt_d[:, 4096:6144])
            gate(mtT, 6144, hA, 4 * NTOK + 10)
            nc.sync.dma_start(mtT[:, 6144:8192], mt_d[:, 6144:8192])
            scoped("Srefl", big_stage, hA, hB, wsT[:])
            for i in reversed(range(3)):
                scoped(f"revb{i}", rev_block, hB, i)

            # tail: out[g-tokens] = sum_kt Mtail_t[kt].T @ h[kt, g-tokens]
            with nc.named_scope("tail"):
                for ch in range(NCH):
                    for gl in range(8):
                        g = ch * 8 + gl
                        gs, ge = g * B, (g + 1) * B
                        ps = psw.tile([DOUT, B], f32, tag="sm")
                        for kt in range(NB):
                            nc.tensor.matmul(
                                ps[:],
                                mtT[:, g * HB + kt * 64:g * HB + (kt + 1) * 64],
                                hB[:, kt * NTOK + gs:kt * NTOK + ge],
                                start=(kt == 0), stop=(kt == NB - 1))
                        nc.vector.tensor_copy(outsb[:, gs:ge], ps[:])
                        if gl % 4 == 3:
                            o0 = ch * 512 + (gl - 3) * B
                            nc.sync.dma_start(out_d[:, o0:o0 + 256],
                                              outsb[:, o0:o0 + 256])

    nc.compile()
    return nc


def _host_weights(Wi, bi, P, rotW, F, G, R, Wo):
    """Fold t-independent weights into the SBUF layouts the kernel expects."""
    import ml_dtypes
    bf16 = ml_dtypes.bfloat16
    W12 = rotW[1] @ rotW[2]
    Srefl = R + R.T
    # jt-major: ws[p, jt*D + kt*128 + m] = Srefl[kt*128+p, jt*128+m]
    ws = Srefl.reshape(NB, 128, NB, 128).transpose(1, 2, 0, 3).reshape(128, NB * D)
    ws = np.ascontiguousarray(ws).astype(bf16)

    # jt-major: wf[p, jt*HB + kt*128 + m] = F[kt*128+p, jt*128+m]
    wf = np.stack([f.reshape(NBH, 128, NBH, 128).transpose(1, 2, 0, 3)
                   .reshape(128, NBH * HB) for f in F]).astype(bf16)
    wg = np.stack([g.reshape(NBH, 128, NBH, 128).transpose(1, 2, 0, 3)
                   .reshape(128, NBH * HB) for g in G]).astype(bf16)

    WpreA = np.concatenate([P @ Wi, (P @ bi)[:, None]], axis=1)  # [D, DIN+1]
    Wpost = P @ Wo.T                                             # [D, DOUT]
    return W12, WpreA, Wpost, ws, wf, wg


def _per_core_mats(c, rotW0, W12, WpreA, Wpost):
    """Per-t folded head/tail matrices for core c, in SBUF layout."""
    import ml_dtypes
    bf16 = ml_dtypes.bfloat16
    ts = [c * TLOC + g for g in range(TLOC)]
    A = np.stack([np.roll(rotW0, (t, t), axis=(0, 1)) for t in ts])  # [16,D,D]
    # Mhead_t = WpreA.T @ A_t @ W12  -> [16, 65, D]
    Mhead = np.matmul(np.matmul(WpreA.T[None], A), W12)
    # Mtail_t = W12.T @ A_t.T @ Wpost -> [16, D, 64]
    Mtail = np.matmul(W12.T[None], np.matmul(A.transpose(0, 2, 1), Wpost))

    # mhead sbuf: [64, g*D + jt*128 + m] = Mhead[g, :64, jt*128+m]
    mh = np.ascontiguousarray(
        Mhead[:, :DIN, :].transpose(1, 0, 2).reshape(DIN, TLOC * D)).astype(bf16)
    # bias: bT[p, g*NB + jt] = Mhead[g, 64, jt*128+p]
    mhb = np.ascontiguousarray(
        Mhead[:, DIN, :].reshape(TLOC, NB, 128).transpose(2, 0, 1)
        .reshape(128, TLOC * NB)).astype(bf16)
    # mtail sbuf: [p, g*HB + kt*64 + m] = Mtail[g, kt*128+p, m]
    mt = np.ascontiguousarray(
        Mtail.reshape(TLOC, NB, 128, DOUT).transpose(2, 0, 1, 3)
        .reshape(128, TLOC * NB * DOUT)).astype(bf16)
    return mh, mhb, mt


def kernel(x, Wi, bi, P, rotW, F, G, R, Wo, bo):
    import ml_dtypes
    bf16 = ml_dtypes.bfloat16
    x = np.asarray(x, np.float32)
    Wi, bi, P = (np.asarray(a, np.float32) for a in (Wi, bi, P))
    rotW, F, G = (np.asarray(a, np.float32) for a in (rotW, F, G))
    R, Wo, bo = (np.asarray(a, np.float32) for a in (R, Wo, bo))

    if "nc" not in _compiled:
        _compiled["nc"] = _build()
    nc = _compiled["nc"]

    W12, WpreA, Wpost, ws, wf, wg = _host_weights(Wi, bi, P, rotW, F, G, R, Wo)

    in_maps = []
    for c in range(NCORES):
        # xt[din, g*B + b] = x[b, c*TLOC + g, din]
        xs = x[:, c * TLOC:(c + 1) * TLOC, :]          # [B, TLOC, DIN]
        xT = xs.transpose(2, 1, 0).reshape(DIN, NTOK)  # [DIN, g*B+b]
        mh, mhb, mt = _per_core_mats(c, rotW[0], W12, WpreA, Wpost)
        in_maps.append({
            "xt": np.ascontiguousarray(xT).astype(bf16),
            "mhead": mh, "mheadb": mhb, "mtail": mt,
            "wf": wf, "wg": wg, "wsrefl": ws,
        })

    from concourse.bass_utils import run_bass_kernel_spmd
    res = run_bass_kernel_spmd(nc, in_maps, list(range(NCORES)))
    _compiled["last_res"] = res

    out = np.empty((B, S, DOUT), np.float32)
    for c in range(NCORES):
        oT = res.results[c]["out"]                     # [DOUT, NTOK]
        out[:, c * TLOC:(c + 1) * TLOC, :] = \
            oT.reshape(DOUT, TLOC, B).transpose(2, 1, 0)
    out += bo.astype(np.float32)
    return out


# revision 33
# speedup vs baseline: 1.0240x; 1.0240x over previous
"""Trainium2 Bass kernel for the Enigma-style CopyMemoryModel.

Math (validated vs reference):
  - The lax.scan carries nothing -> every timestep t is independent.
  - t < 128 and d = 1024  =>  rotors 1,2 have pos = 0 (no roll); only rotor 0
    rolls by t, and roll(roll(h,-t) @ W, t) == h @ roll(W, (t,t), (0,1)).
  - Everything before the first rev block is LINEAR with the only t-dependence
    being rotor 0's roll -> fold on host into per-t head matrices
        Mhead_t = [P@Wi | P@bi].T @ roll(rotW0,(t,t)) @ (rotW1@rotW2)   [65,1024]
  - Everything after the last rev block is linear too -> per-t tail matrices
        Mtail_t = (rotW1@rotW2).T @ roll(rotW0,(t,t)).T @ (P@Wo.T)      [1024,64]
  - On chip only:  head (per-t small matmuls), 6 rev couplings fwd,
    Srefl = R+R.T big stage, 6 rev couplings bwd, tail.  bo added on host.
  - Layout on chip: activations stored transposed, hT[128 part, 8 blocks x 1024
    tokens] per core; every stage is out_block[jt] = sum_kt W[kt,jt].T @ h[kt].
  - bf16 datapath (PSUM accumulation fp32, final output fp32).

Perf structure (v2):
  - All input DMAs hoisted to kernel start, spread over 4 issue queues in
    consumption order: scalar{xt,wf0,wg0,wf1,wg1}, sync{mh even, ws x4},
    vector{mh 1,3,5,7}, gpsimd{mhb, mh 9,11,13,15, wf2, wg2, mt x4}.
    In-queue ordering delays ws/mt transfers behind the critical head DMAs.
  - Srefl weights stored jt-major so each output block jt only needs its
    1024-col slice -> Srefl can start before the full 2MB matrix lands.
  - Small junk-matmul warmup keeps the PE p-state ramp warm while the first
    DMAs land.

Sharding: time-sharded; core c handles t in [c*16, (c+1)*16), all 64 batch
samples -> 1024 tokens per core, token column = g*64 + b.
"""
import numpy as np

B, S, DIN, D, DOUT = 64, 128, 64, 1024, 64
NCORES = 8
TLOC = S // NCORES          # 16 timesteps per core
NTOK = B * TLOC             # 1024 tokens per core
NB = D // 128               # 8 row blocks
NCH = NTOK // 512           # 2 column chunks of 512
HB = 512                    # half of D (rev-block split)
NBH = HB // 128             # 4 blocks per half

_compiled = {}


def _build():
    import concourse.bacc as bacc
    import concourse.mybir as mybir
    from concourse.tile import TileContext

    f32 = mybir.dt.float32
    bf16 = mybir.dt.bfloat16
    ACT_TANH = mybir.ActivationFunctionType.Tanh
    ACT_COPY = mybir.ActivationFunctionType.Copy

    nc = bacc.Bacc(None, target_bir_lowering=False, debug=True)

    xt_d = nc.dram_tensor("xt", [DIN, NTOK], bf16, kind="ExternalInput")
    mh_d = nc.dram_tensor("mhead", [DIN, TLOC * D], bf16, kind="ExternalInput")
    mhb_d = nc.dram_tensor("mheadb", [128, TLOC * NB], bf16, kind="ExternalInput")
    wf_d = nc.dram_tensor("wf", [3, 128, NBH * HB], bf16, kind="ExternalInput")
    wg_d = nc.dram_tensor("wg", [3, 128, NBH * HB], bf16, kind="ExternalInput")
    ws_d = nc.dram_tensor("wsrefl", [128, NB * D], bf16, kind="ExternalInput")
    mt_d = nc.dram_tensor("mtail", [128, TLOC * HB], bf16, kind="ExternalInput")
    out_d = nc.dram_tensor("out", [DOUT, NTOK], f32, kind="ExternalOutput")

    with TileContext(nc) as tc:
        with (
            tc.tile_pool(name="hbuf", bufs=1) as hpool,
            tc.tile_pool(name="wpool", bufs=1) as wpool,
            tc.tile_pool(name="fgpool", bufs=1) as fgpool,
            tc.tile_pool(name="hdpool", bufs=1) as hdpool,
            tc.tile_pool(name="cpool", bufs=1) as cpool,
            tc.tile_pool(name="tpool", bufs=3) as tpool,
            tc.tile_pool(name="ps1", bufs=6, space="PSUM") as ps1,
            tc.tile_pool(name="psw", bufs=2, space="PSUM") as psw,
        ):
            hA = hpool.tile([128, NB * NTOK], bf16)
            hB = hpool.tile([128, NB * NTOK], bf16)
            hAR = hA[:].rearrange("p (n t) -> p n t", n=NB)

            xt = cpool.tile([DIN, NTOK], bf16)
            bT = cpool.tile([128, TLOC * NB], bf16)
            outsb = cpool.tile([DOUT, NTOK], f32)

            junk = cpool.tile([128, 256], bf16)
            nc.gpsimd.memset(junk[:], 0.0)

            def junk_mm(n, gate=None):
                for r in range(n):
                    wps = psw.tile([128, 256], f32, tag="sm")
                    if r == 0 and gate is not None:
                        # pace the PE start to DMA supply: first junk reads a
                        # mid-schedule DMA's landing zone, so the PE starts
                        # late but then runs gap-free (keeps p-state high)
                        nc.tensor.matmul(wps[:], junk[0:64, 0:128],
                                         gate, start=True, stop=True)
                    else:
                        nc.tensor.matmul(wps[:], junk[:, 0:128],
                                         junk[:], start=True, stop=True)

            # ---- hoisted input DMAs, 4 issue queues, consumption order ----
            mhall = hdpool.tile([DIN, TLOC * D], bf16)
            wfT = [fgpool.tile([128, NBH * HB], bf16, tag=f"wf{i}",
                               name=f"wf{i}") for i in range(3)]
            wgT = [fgpool.tile([128, NBH * HB], bf16, tag=f"wg{i}",
                               name=f"wg{i}") for i in range(3)]
            wsT = wpool.tile([128, NB * D], bf16, tag="ws")
            mtT = wpool.tile([128, TLOC * HB], bf16, tag="mt")

            # ~128-256KB pieces over the 3 DGE queues in consumption order.
            # Only the head-critical set issues up front; everything needed
            # later is gated behind compute milestones (see `gate` below) so
            # the scheduler cannot hoist it into the critical window.
            def mh_piece(q, g):
                q.dma_start(mhall[0:DIN, g * D:(g + 1) * D],
                            mh_d[:, g * D:(g + 1) * D])

            def half(q, dst, src, h):
                q.dma_start(dst[:, h * 1024:(h + 1) * 1024],
                            src[:, h * 1024:(h + 1) * 1024])

            # issue the head-critical pieces round-robin across the 3 queues
            # in exact consumption order
            qs = [nc.sync, nc.scalar, nc.gpsimd]
            order = (["bT", "xt", 0, 1, 2, 3, "wf0a", "wf0b", 4, 5, 6, 7,
                      "wg0a", "wg0b", 8, 9, 10, 11, 12, 13, 14, 15])
            for i, item in enumerate(order):
                q = qs[i % 3]
                if item == "bT":
                    q.dma_start(bT[:], mhb_d[:])
                elif item == "xt":
                    q.dma_start(xt[:], xt_d[:])
                elif item == "wf0a":
                    half(q, wfT[0][:], wf_d[0], 0)
                elif item == "wf0b":
                    half(q, wfT[0][:], wf_d[0], 1)
                elif item == "wg0a":
                    half(q, wgT[0][:], wg_d[0], 0)
                elif item == "wg0b":
                    half(q, wgT[0][:], wg_d[0], 1)
                else:
                    mh_piece(q, item)

            def gate(dst_tile, col, src_tile, scol):
                # tiny WAW anchor: the following DMA into dst_tile[:, col:...]
                # cannot issue until src_tile[0, scol:scol+2] is final
                nc.gpsimd.tensor_copy(dst_tile[0:1, col:col + 2],
                                      src_tile[0:1, scol:scol + 2])

            with nc.named_scope("warmup"):
                junk_mm(4, gate=mhall[0:64, 0:256])

            # head: h[jt-block, g-tokens] = Mhead_t[:, jt].T @ x[g-tokens] + b
            def head_group(g):
                mo = g * D
                gs, ge = g * B, (g + 1) * B
                ps = ps1.tile([128, 512], f32)
                for jt in range(NB):
                    nc.tensor.matmul(ps[:, jt * 64:(jt + 1) * 64],
                                     mhall[:, mo + jt * 128:mo + (jt + 1) * 128],
                                     xt[:, gs:ge], start=True, stop=True)
                psR = ps[:].rearrange("p (n t) -> p n t", n=NB)
                bias = bT[:, g * NB:(g + 1) * NB].broadcast_to([128, NB, B])
                nc.vector.tensor_tensor(hAR[:, :, gs:ge], psR, bias,
                                        op=mybir.AluOpType.add)

            def big_stage(src, dst, w):
                # jt-major weight layout: w[:, jt*D + kt*128 + m]
                for ch in range(NCH):
                    for jt in range(NB):
                        ps = ps1.tile([128, 512], f32)
                        for kt in range(NB):
                            nc.tensor.matmul(
                                ps[:],
                                w[:, jt * D + kt * 128:jt * D + (kt + 1) * 128],
                                src[:, kt * NTOK + ch * 512:kt * NTOK + (ch + 1) * 512],
                                start=(kt == 0), stop=(kt == NB - 1),
                            )
                        nc.scalar.activation(
                            dst[:, jt * NTOK + ch * 512:jt * NTOK + (ch + 1) * 512],
                            ps[:], ACT_COPY)

            def coupling_chunk(buf, fg, in_half, out_half, ch):
                # buf[out_half] += tanh(W.T @ buf[in_half]) for token chunk ch
                # jt-major weight layout: fg[:, jt*HB + kt*128 + m]
                for jt in range(NBH):
                    ps = ps1.tile([128, 512], f32)
                    for kt in range(NBH):
                        nc.tensor.matmul(
                            ps[:],
                            fg[:, jt * HB + kt * 128:jt * HB + (kt + 1) * 128],
                            buf[:, (in_half * NBH + kt) * NTOK + ch * 512:
                                (in_half * NBH + kt) * NTOK + (ch + 1) * 512],
                            start=(kt == 0), stop=(kt == NBH - 1),
                        )
                    tmp = tpool.tile([128, 512], bf16)
                    nc.scalar.activation(tmp[:], ps[:], ACT_TANH)
                    dsl = buf[:, (out_half * NBH + jt) * NTOK + ch * 512:
                              (out_half * NBH + jt) * NTOK + (ch + 1) * 512]
                    nc.vector.tensor_add(dsl, dsl, tmp[:])

            def coupling(buf, fg, in_half, out_half):
                for ch in range(NCH):
                    coupling_chunk(buf, fg, in_half, out_half, ch)

            def cseg(buf, fg, in_half, out_half, seg):
                # one 256-token segment of a coupling (head-phase pipelining)
                for jt in range(NBH):
                    ps = ps1.tile([128, 256], f32)
                    for kt in range(NBH):
                        nc.tensor.matmul(
                            ps[:],
                            fg[:, jt * HB + kt * 128:jt * HB + (kt + 1) * 128],
                            buf[:, (in_half * NBH + kt) * NTOK + seg * 256:
                                (in_half * NBH + kt) * NTOK + (seg + 1) * 256],
                            start=(kt == 0), stop=(kt == NBH - 1),
                        )
                    tmp = tpool.tile([128, 256], bf16)
                    nc.scalar.activation(tmp[:], ps[:], ACT_TANH)
                    dsl = buf[:, (out_half * NBH + jt) * NTOK + seg * 256:
                              (out_half * NBH + jt) * NTOK + (seg + 1) * 256]
                    nc.vector.tensor_add(dsl, dsl, tmp[:])

            def rev_block(buf, i):
                coupling(buf, wfT[i][:], in_half=1, out_half=0)
                coupling(buf, wgT[i][:], in_half=0, out_half=1)

            def scoped(name, fn, *args, **kw):
                with nc.named_scope(name):
                    fn(*args, **kw)

            # heads and revf0 interleaved in 256-token segments, in the same
            # order the DMA pieces land — PE does coupling work on early
            # groups while late groups stream in
            with nc.named_scope("head"):
                for g in range(4):
                    head_group(g)
            scoped("f0s0", cseg, hA, wfT[0][:], 1, 0, 0)
            with nc.named_scope("head2"):
                for g in range(4, 8):
                    head_group(g)
            scoped("f0s1", cseg, hA, wfT[0][:], 1, 0, 1)
            scoped("g0s0", cseg, hA, wgT[0][:], 0, 1, 0)
            # wf1 after g0s0 (anchor: hA half1 seg0)
            gate(wfT[1], 0, hA, 4 * NTOK + 20)
            half(nc.sync, wfT[1][:], wf_d[1], 0)
            gate(wfT[1], 1024, hA, 4 * NTOK + 22)
            half(nc.gpsimd, wfT[1][:], wf_d[1], 1)
            junk_mm(2)
            with nc.named_scope("head3"):
                for g in range(8, 12):
                    head_group(g)
            scoped("f0s2", cseg, hA, wfT[0][:], 1, 0, 2)
            scoped("g0s1", cseg, hA, wgT[0][:], 0, 1, 1)
            # wg1 after g0s1 (anchor: hA half1 seg1)
            gate(wgT[1], 0, hA, 4 * NTOK + 276)
            half(nc.sync, wgT[1][:], wg_d[1], 0)
            gate(wgT[1], 1024, hA, 4 * NTOK + 278)
            half(nc.gpsimd, wgT[1][:], wg_d[1], 1)
            with nc.named_scope("head4"):
                for g in range(12, TLOC):
                    head_group(g)
            scoped("f0s3", cseg, hA, wfT[0][:], 1, 0, 3)
            scoped("g0s2", cseg, hA, wgT[0][:], 0, 1, 2)
            scoped("g0s3", cseg, hA, wgT[0][:], 0, 1, 3)
            # wf2 after g0s3 (anchor: hA half1 seg3)
            gate(wfT[2], 0, hA, 4 * NTOK + 788)
            half(nc.sync, wfT[2][:], wf_d[2], 0)
            gate(wfT[2], 1024, hA, 4 * NTOK + 790)
            half(nc.gpsimd, wfT[2][:], wf_d[2], 1)
            scoped("revf1F", coupling, hA, wfT[1][:], 1, 0)
            # wg2 + ws q0 after revf1F
            gate(wgT[2], 0, hA, 4)
            half(nc.sync, wgT[2][:], wg_d[2], 0)
            gate(wgT[2], 1024, hA, 6)
            half(nc.gpsimd, wgT[2][:], wg_d[2], 1)
            gate(wsT, 0, hA, 8)
            nc.sync.dma_start(wsT[:, 0:2048], ws_d[:, 0:2048])
            scoped("revf1G", coupling, hA, wgT[1][:], 0, 1)
            # ws q1/q2 after revf1G
            gate(wsT, 2048, hA, 4 * NTOK + 4)
            nc.sync.dma_start(wsT[:, 2048:4096], ws_d[:, 2048:4096])
            gate(wsT, 4096, hA, 4 * NTOK + 6)
            nc.gpsimd.dma_start(wsT[:, 4096:6144], ws_d[:, 4096:6144])
            scoped("revf2F", coupling, hA, wfT[2][:], 1, 0)
            # ws q3 + mt q0/q1 after revf2F
            gate(wsT, 6144, hA, 10)
            nc.sync.dma_start(wsT[:, 6144:8192], ws_d[:, 6144:8192])
            gate(mtT, 0, hA, 12)
            nc.gpsimd.dma_start(mtT[:, 0:2048], mt_d[:, 0:2048])
            gate(mtT, 2048, hA, 14)
            nc.sync.dma_start(mtT[:, 2048:4096], mt_d[:, 2048:4096])
            scoped("revf2G", coupling, hA, wgT[2][:], 0, 1)
            # mt q2/q3 after revf2G
            gate(mtT, 4096, hA, 4 * NTOK + 8)
            nc.gpsimd.dma_start(mtT[:, 4096:6144], mt_d[:, 4096:6144])
            gate(mtT, 6144, hA, 4 * NTOK + 10)
            nc.sync.dma_start(mtT[:, 6144:8192], mt_d[:, 6144:8192])
            scoped("Srefl", big_stage, hA, hB, wsT[:])
            for i in reversed(range(3)):
                scoped(f"revb{i}", rev_block, hB, i)

            # tail: out[g-tokens] = sum_kt Mtail_t[kt].T @ h[kt, g-tokens]
            with nc.named_scope("tail"):
                for ch in range(NCH):
                    for gl in range(8):
                        g = ch * 8 + gl
                        gs, ge = g * B, (g + 1) * B
                        ps = psw.tile([DOUT, B], f32, tag="sm")
                        for kt in range(NB):
                            nc.tensor.matmul(
                                ps[:],
                                mtT[:, g * HB + kt * 64:g * HB + (kt + 1) * 64],
                                hB[:, kt * NTOK + gs:kt * NTOK + ge],
                                start=(kt == 0), stop=(kt == NB - 1))
                        nc.vector.tensor_copy(outsb[:, gs:ge], ps[:])
                        if gl % 4 == 3:
                            o0 = ch * 512 + (gl - 3) * B
                            nc.sync.dma_start(out_d[:, o0:o0 + 256],
                                              outsb[:, o0:o0 + 256])

    nc.compile()
    return nc


def _host_weights(Wi, bi, P, rotW, F, G, R, Wo):
    """Fold t-independent weights into the SBUF layouts the kernel expects."""
    import ml_dtypes
    bf16 = ml_dtypes.bfloat16
    W12 = rotW[1] @ rotW[2]
    Srefl = R + R.T
    # jt-major: ws[p, jt*D + kt*128 + m] = Srefl[kt*128+p, jt*128+m]
    ws = Srefl.reshape(NB, 128, NB, 128).transpose(1, 2, 0, 3).reshape(128, NB * D)
    ws = np.ascontiguousarray(ws).astype(bf16)

    # jt-major: wf[p, jt*HB + kt*128 + m] = F[kt*128+p, jt*128+m]
    wf = np.stack([f.reshape(NBH, 128, NBH, 128).transpose(1, 2, 0, 3)
                   .reshape(128, NBH * HB) for f in F]).astype(bf16)
    wg = np.stack([g.reshape(NBH, 128, NBH, 128).transpose(1, 2, 0, 3)
                   .reshape(128, NBH * HB) for g in G]).astype(bf16)

    WpreA = np.concatenate([P @ Wi, (P @ bi)[:, None]], axis=1)  # [D, DIN+1]
    Wpost = P @ Wo.T                                             # [D, DOUT]
    return W12, WpreA, Wpost, ws, wf, wg


def _per_core_mats(c, rotW0, W12, WpreA, Wpost):
    """Per-t folded head/tail matrices for core c, in SBUF layout."""
    import ml_dtypes
    bf16 = ml_dtypes.bfloat16
    ts = [c * TLOC + g for g in range(TLOC)]
    A = np.stack([np.roll(rotW0, (t, t), axis=(0, 1)) for t in ts])  # [16,D,D]
    # Mhead_t = WpreA.T @ A_t @ W12  -> [16, 65, D]
    Mhead = np.matmul(np.matmul(WpreA.T[None], A), W12)
    # Mtail_t = W12.T @ A_t.T @ Wpost -> [16, D, 64]
    Mtail = np.matmul(W12.T[None], np.matmul(A.transpose(0, 2, 1), Wpost))

    # mhead sbuf: [64, g*D + jt*128 + m] = Mhead[g, :64, jt*128+m]
    mh = np.ascontiguousarray(
        Mhead[:, :DIN, :].transpose(1, 0, 2).reshape(DIN, TLOC * D)).astype(bf16)
    # bias: bT[p, g*NB + jt] = Mhead[g, 64, jt*128+p]
    mhb = np.ascontiguousarray(
        Mhead[:, DIN, :].reshape(TLOC, NB, 128).transpose(2, 0, 1)
        .reshape(128, TLOC * NB)).astype(bf16)
    # mtail sbuf: [p, g*HB + kt*64 + m] = Mtail[g, kt*128+p, m]
    mt = np.ascontiguousarray(
        Mtail.reshape(TLOC, NB, 128, DOUT).transpose(2, 0, 1, 3)
        .reshape(128, TLOC * NB * DOUT)).astype(bf16)
    return mh, mhb, mt


def kernel(x, Wi, bi, P, rotW, F, G, R, Wo, bo):
    import ml_dtypes
    bf16 = ml_dtypes.bfloat16
    x = np.asarray(x, np.float32)
    Wi, bi, P = (np.asarray(a, np.float32) for a in (Wi, bi, P))
    rotW, F, G = (np.asarray(a, np.float32) for a in (rotW, F, G))
    R, Wo, bo = (np.asarray(a, np.float32) for a in (R, Wo, bo))

    if "nc" not in _compiled:
        _compiled["nc"] = _build()
    nc = _compiled["nc"]

    W12, WpreA, Wpost, ws, wf, wg = _host_weights(Wi, bi, P, rotW, F, G, R, Wo)

    in_maps = []
    for c in range(NCORES):
        # xt[din, g*B + b] = x[b, c*TLOC + g, din]
        xs = x[:, c * TLOC:(c + 1) * TLOC, :]          # [B, TLOC, DIN]
        xT = xs.transpose(2, 1, 0).reshape(DIN, NTOK)  # [DIN, g*B+b]
        mh, mhb, mt = _per_core_mats(c, rotW[0], W12, WpreA, Wpost)
        in_maps.append({
            "xt": np.ascontiguousarray(xT).astype(bf16),
            "mhead": mh, "mheadb": mhb, "mtail": mt,
            "wf": wf, "wg": wg, "wsrefl": ws,
        })

    from concourse.bass_utils import run_bass_kernel_spmd
    res = run_bass_kernel_spmd(nc, in_maps, list(range(NCORES)))
    _compiled["last_res"] = res

    out = np.empty((B, S, DOUT), np.float32)
    for c in range(NCORES):
        oT = res.results[c]["out"]                     # [DOUT, NTOK]
        out[:, c * TLOC:(c + 1) * TLOC, :] = \
            oT.reshape(DOUT, TLOC, B).transpose(2, 1, 0)
    out += bo.astype(np.float32)
    return out
